# revision 14
# baseline (speedup 1.0000x reference)
"""Trainium2 Bass kernel for nn_Block_70944269795510 (involution block).

8 NeuronCores, data-parallel over batch (2 samples/core). Per sample:
  conv1 (PE bf16) -> tanh/bn1 (ACT) into padded-Y bf16 (ypa) + one-element
      shifted parity copy (ypb; keeps bf16 DVE reads 4B-aligned for odd
      kernel-column offsets)
  red (PE bf16) -> relu (ACT) -> rt bf16 [64, HW]
  involution in 4 pixel-chunks of 784 (14 rows), m-serialized, with the
      tap-accumulator living in PSUM:
        per tap: span matmul (PE, 64-contr, 784 cols) -> PSUM
                 evac+bias (ACT identity) -> wd bf16 -> mul wd*ysh (DVE 2x)
                 [STT taps skip the evac: (ps+bias)*ysh via DVE stt at 1x]
                 identity-matmul accumulate pr into acc-PSUM (PE)
  tanh/bn2 (ACT, from acc-PSUM) -> y2 -> conv3 (PE bf16) -> bn3 (ACT)
      -> +x (DVE) -> DMA out

The conv phases of neighbouring samples are software-pipelined into the
involution chunk stream (issued as filler units between taps) so the PE
queue never drains at sample seams - keeping the DVFS p-state high.
"""

import sys

for _p in ("/opt/trn_rl_repo", "/root/.axon_site/_ro/trn_rl_repo"):
    if _p not in sys.path:
        sys.path.append(_p)

import numpy as np
import ml_dtypes
from contextlib import ExitStack

import concourse.bass as bass
import concourse.mybir as mybir
from concourse import bacc
from concourse.tile import TileContext
from concourse.bass_utils import run_bass_kernel_spmd

F32 = mybir.dt.float32
BF16 = mybir.dt.bfloat16
AF = mybir.ActivationFunctionType
ALU = mybir.AluOpType

B, C, H, W = 16, 256, 56, 56
HW = H * W
KK = 7
G, GC, RED = 16, 16, 64
EPS = 1e-5
PAD = 3
N_CORES = 8
S_PER_CORE = B // N_CORES
PW = 64
HP = H + 2 * PAD
NPAD = HP * PW

T448 = 448
CH = 784            # involution pixel-chunk (14 rows of 56)
NCH = 4             # chunks per image
CROWS = 14

# taps whose product is computed by DVE scalar_tensor_tensor straight from
# PSUM (skipping the ACT evacuation); tunable for ACT/DVE balance
STT = frozenset({1, 4, 8, 11, 15, 18, 22, 25, 29, 32, 36, 39, 43, 46})
# taps accumulated on DVE into an SBUF bf16 chain (merged into acc-PSUM at
# the end) instead of PE identity-adds; relieves the PE column budget
DVE_ADD = (0, 6, 13, 19, 26, 33, 40, 47)


def _build_nc():
    nc = bacc.Bacc("TRN2", target_bir_lowering=False, debug=False)

    xd = nc.dram_tensor("x", [S_PER_CORE, 2, 128, HW], F32, kind="ExternalInput").ap()
    xbd = nc.dram_tensor("xb", [S_PER_CORE, 2, 128, HW], BF16, kind="ExternalInput").ap()
    w1d = nc.dram_tensor("w1t", [128, 2, 256], BF16, kind="ExternalInput").ap()
    rwd = nc.dram_tensor("rwt", [128, 2, 64], BF16, kind="ExternalInput").ap()
    spd = nc.dram_tensor("srep", [64, 2, 49 * 128], BF16, kind="ExternalInput").ap()
    brd = nc.dram_tensor("brep", [128, 2, 49], F32, kind="ExternalInput").ap()
    w3d = nc.dram_tensor("w3t", [128, 2, 256], BF16, kind="ExternalInput").ap()
    scd = nc.dram_tensor("scal", [128, 2, 6], F32, kind="ExternalInput").ap()
    srcd = nc.dram_tensor("scred", [128, 2], F32, kind="ExternalInput").ap()
    idd = nc.dram_tensor("ident", [128, 128], BF16, kind="ExternalInput").ap()
    outd = nc.dram_tensor("out", [S_PER_CORE, 2, 128, HW], F32, kind="ExternalOutput").ap()

    with TileContext(nc) as tc, ExitStack() as ctx:
        consts = ctx.enter_context(tc.tile_pool(name="consts", bufs=1))
        xpool = ctx.enter_context(tc.tile_pool(name="xp", bufs=2))
        xbpool = ctx.enter_context(tc.tile_pool(name="xb", bufs=2))
        ypapool = ctx.enter_context(tc.tile_pool(name="ypa", bufs=4))
        ypbpool = ctx.enter_context(tc.tile_pool(name="ypb", bufs=4))
        rpool = ctx.enter_context(tc.tile_pool(name="rp", bufs=2))
        prpool = ctx.enter_context(tc.tile_pool(name="prp", bufs=10))
        y2pool = ctx.enter_context(tc.tile_pool(name="y2p", bufs=4))
        opool = ctx.enter_context(tc.tile_pool(name="op", bufs=1))
        pspool = ctx.enter_context(tc.tile_pool(name="psp", bufs=1, space="PSUM"))

        w1t = consts.tile([128, 2, 256], BF16)
        nc.sync.dma_start(out=w1t, in_=w1d)
        rwt = consts.tile([128, 2, 64], BF16)
        nc.sync.dma_start(out=rwt, in_=rwd)
        srep = consts.tile([64, 2, 49 * 128], BF16)
        nc.sync.dma_start(out=srep, in_=spd)
        brep = consts.tile([128, 2, 49], F32)
        nc.sync.dma_start(out=brep, in_=brd)
        w3t = consts.tile([128, 2, 256], BF16)
        nc.sync.dma_start(out=w3t, in_=w3d)
        scal = consts.tile([128, 2, 6], F32)
        nc.sync.dma_start(out=scal, in_=scd)
        scred = consts.tile([128, 2], F32)
        nc.sync.dma_start(out=scred, in_=srcd)
        ident = consts.tile([128, 128], BF16)
        nc.sync.dma_start(out=ident, in_=idd)

        # 448-col tile pairs packed into [128,1024] psum tiles (512-stripes)
        pairs = [(0, 2), (2, 2), (4, 2), (6, 1)]

        # per-sample state
        ST = [dict() for _ in range(S_PER_CORE)]

        def emit_load(s):
            st = ST[s]
            st["xbs"] = []
            for m in range(2):
                xbt = xbpool.tile([128, HW], BF16, tag="xb", name=f"xb{s}{m}")
                nc.sync.dma_start(out=xbt, in_=xbd[s, m])
                st["xbs"].append(xbt)

        def emit_load_x(s):
            st = ST[s]
            st["xs"] = []
            for m in range(2):
                xt = xpool.tile([128, HW], F32, tag="x", name=f"x{s}{m}")
                nc.sync.dma_start(out=xt, in_=xd[s, m])
                st["xs"].append(xt)

        def emit_conv1_start(s, m):
            st = ST[s]
            if "ypa" not in st:
                st["ypa"], st["ypb"] = [], []
            yat = ypapool.tile([128, NPAD], BF16, tag="ypa", name=f"ya{s}{m}")
            ybt = ypbpool.tile([128, NPAD], BF16, tag="ypb", name=f"yb{s}{m}")
            nc.vector.memset(yat[:], 0.0)
            st["ypa"].append(yat)
            st["ypb"].append(ybt)

        def emit_conv1_pair(s, m, t0, cnt):
            st = ST[s]
            yat = st["ypa"][m]
            yav = yat[:].rearrange("p (h w) -> p h w", w=PW)
            ps = pspool.tile([128, 1024], F32, tag="pss", bufs=3, name=f"c1p{s}{m}{t0}")
            psv = ps[:].rearrange("p (t w) -> p t w", w=512)
            for j in range(cnt):
                t = t0 + j
                for k in range(2):
                    nc.tensor.matmul(
                        psv[:, j, 0:T448],
                        w1t[:, k, m * 128:(m + 1) * 128],
                        st["xbs"][k][:, t * T448:(t + 1) * T448],
                        start=(k == 0), stop=(k == 1),
                    )
            psq = psv[:, 0:cnt, 0:T448].rearrange("p t (r w) -> p t r w", w=W)
            nc.scalar.activation(
                out=yav[:, PAD + 8 * t0:PAD + 8 * (t0 + cnt), PAD:PAD + W]
                .rearrange("p (t r) w -> p t r w", r=8),
                in_=psq,
                func=AF.Tanh,
                scale=scal[:, m, 0:1],
                bias=scal[:, m, 1:2],
            )

        def emit_conv1_end(s, m):
            st = ST[s]
            nc.vector.tensor_copy(st["ypb"][m][:, 0:NPAD - 1], st["ypa"][m][:, 1:NPAD])

        def emit_red_start(s):
            ST[s]["rt"] = rpool.tile([64, HW], BF16, tag="r", name=f"r{s}")

        def emit_red_pair(s, t0, cnt):
            st = ST[s]
            rv = st["rt"][:].rearrange("p (t w) -> p t w", w=T448)
            ps = pspool.tile([128, 1024], F32, tag="pss", bufs=3, name=f"rp{s}{t0}")
            psv = ps[:].rearrange("p (t w) -> p t w", w=512)
            for j in range(cnt):
                t = t0 + j
                for k in range(2):
                    yk = st["ypa"][k][:].rearrange("p (h w) -> p h w", w=PW)
                    rhs = yk[:, PAD + 8 * t:PAD + 8 * (t + 1), PAD:PAD + W]
                    nc.tensor.matmul(
                        psv[0:64, j, 0:T448],
                        rwt[:, k, :],
                        rhs,
                        start=(k == 0), stop=(k == 1),
                    )
            nc.scalar.activation(
                out=rv[:, t0:t0 + cnt, :],
                in_=psv[0:64, 0:cnt, 0:T448],
                func=AF.Relu,
                scale=scred[0:64, 0:1],
                bias=scred[0:64, 1:2],
            )

        def emit_inv_chunk(s, cix, m, fillers):
            """49-tap involution for one (pixel-chunk, m); pops filler
            closures after taps 10/22/34/46 to keep other phases flowing."""
            st = ST[s]
            if "y2" not in st:
                st["y2"] = [
                    y2pool.tile([128, HW], BF16, tag="y2", name=f"y2{s}{mm}")
                    for mm in range(2)
                ]
            c0 = cix * CH
            r0c = cix * CROWS
            yav = st["ypa"][m][:].rearrange("p (h w) -> p h w", w=PW)
            ybv = st["ypb"][m][:].rearrange("p (h w) -> p h w", w=PW)
            rt = st["rt"]
            acc = pspool.tile([128, 1024], F32, tag="acc", bufs=1, name=f"acc{s}{cix}{m}")
            acc_sb = prpool.tile([128, CH], BF16, tag="accsb", bufs=2, name=f"as{s}{cix}{m}")
            pend = {}
            pe_taps = [p for p in range(49) if p not in DVE_ADD]

            def flush_add(k):
                if k in DVE_ADD:
                    if k != DVE_ADD[0]:
                        nc.vector.tensor_add(acc_sb[:], acc_sb[:], pend.pop(k)[:])
                    return
                pr = pend.pop(k)
                first = k == pe_taps[0]
                nc.tensor.matmul(
                    acc[:, 0:512], ident[:], pr[:, 0:512],
                    start=first, stop=False,
                )
                nc.tensor.matmul(
                    acc[:, 512:CH], ident[:], pr[:, 512:CH],
                    start=first, stop=False,
                )

            def emit_tap(p):
                di, dj = p // KK, p % KK
                ps = pspool.tile([128, 1024], F32, tag="pss", bufs=3, name=f"sp{s}{cix}{m}{p}")
                for off, n in ((0, 512), (512, CH - 512)):
                    nc.tensor.matmul(
                        ps[:, off:off + n],
                        srep[:, m, p * 128:(p + 1) * 128],
                        rt[:, c0 + off:c0 + off + n],
                        start=True, stop=True,
                    )
                if dj % 2 == 0:
                    ysh = yav[:, r0c + di:r0c + di + CROWS, dj:dj + W]
                else:
                    ysh = ybv[:, r0c + di:r0c + di + CROWS, dj - 1:dj - 1 + W]
                if p == DVE_ADD[0]:
                    pr = acc_sb
                else:
                    pr = prpool.tile([128, CH], BF16, tag="pr", name=f"pr{s}{cix}{m}{p}")
                if p in STT:
                    nc.vector.scalar_tensor_tensor(
                        out=pr[:].rearrange("p (h w) -> p h w", w=W),
                        in0=ps[:, 0:CH].rearrange("p (h w) -> p h w", w=W),
                        scalar=brep[:, m, p:p + 1],
                        in1=ysh,
                        op0=ALU.add,
                        op1=ALU.mult,
                    )
                else:
                    nc.scalar.activation(
                        out=pr[:],
                        in_=ps[:, 0:CH],
                        func=AF.Identity,
                        scale=1.0,
                        bias=brep[:, m, p:p + 1],
                    )
                    nc.vector.tensor_mul(
                        pr[:].rearrange("p (h w) -> p h w", w=W),
                        pr[:].rearrange("p (h w) -> p h w", w=W),
                        ysh,
                    )
                pend[p] = pr

            # batches of 3 taps: spans together, trailing adds together --
            # each srep<->ident weight switch costs ~105ns of PE drain
            done = 0
            for b in range(0, 51, 3):
                for p in range(b, min(b + 3, 49)):
                    emit_tap(p)
                while done < min(b, 49):
                    flush_add(done)
                    done += 1
                if b in (12, 24, 36, 48) and fillers:
                    fillers.pop(0)()
            while done < 49:
                flush_add(done)
                done += 1
            for off, n, last in ((0, 512, False), (512, CH - 512, True)):
                nc.tensor.matmul(
                    acc[:, off:off + n], ident[:], acc_sb[:, off:off + n],
                    start=False, stop=True,
                )
            nc.scalar.activation(
                out=st["y2"][m][:, c0:c0 + CH],
                in_=acc[:, 0:CH],
                func=AF.Tanh,
                scale=scal[:, m, 2:3],
                bias=scal[:, m, 3:4],
            )

        def emit_conv3_start(s, m):
            st = ST[s]
            if "ot" not in st:
                st["ot"] = {}
            st["ot"][m] = opool.tile([128, HW], F32, tag="o", name=f"o{s}{m}")

        def emit_conv3_pair(s, m, t0, cnt):
            st = ST[s]
            ot = st["ot"][m]
            ov = ot[:].rearrange("p (t w) -> p t w", w=T448)
            ps = pspool.tile([128, 1024], F32, tag="pss", bufs=3, name=f"c3p{s}{m}{t0}")
            psv = ps[:].rearrange("p (t w) -> p t w", w=512)
            for j in range(cnt):
                t = t0 + j
                for k in range(2):
                    nc.tensor.matmul(
                        psv[:, j, 0:T448],
                        w3t[:, k, m * 128:(m + 1) * 128],
                        st["y2"][k][:, t * T448:(t + 1) * T448],
                        start=(k == 0), stop=(k == 1),
                    )
            nc.scalar.activation(
                out=ov[:, t0:t0 + cnt, :],
                in_=psv[:, 0:cnt, 0:T448],
                func=AF.Identity,
                scale=scal[:, m, 4:5],
                bias=scal[:, m, 5:6],
            )

        def emit_conv3_end(s, m):
            st = ST[s]
            ot = st["ot"][m]
            nc.vector.tensor_add(ot[:], ot[:], st["xs"][m][:])
            nc.sync.dma_start(out=outd[s, m], in_=ot[:])

        def conv1_units(s):
            units = []
            for m in range(2):
                units.append(lambda s=s, m=m: emit_conv1_start(s, m))
                for t0, cnt in pairs:
                    units.append(lambda s=s, m=m, t0=t0, cnt=cnt: emit_conv1_pair(s, m, t0, cnt))
                units.append(lambda s=s, m=m: emit_conv1_end(s, m))
            return units

        def red_units(s):
            units = [lambda s=s: emit_red_start(s)]
            for t0, cnt in pairs:
                units.append(lambda s=s, t0=t0, cnt=cnt: emit_red_pair(s, t0, cnt))
            return units

        def conv3_units(s):
            units = []
            for m in range(2):
                units.append(lambda s=s, m=m: emit_conv3_start(s, m))
                for t0, cnt in pairs:
                    units.append(lambda s=s, m=m, t0=t0, cnt=cnt: emit_conv3_pair(s, m, t0, cnt))
                units.append(lambda s=s, m=m: emit_conv3_end(s, m))
                if s == 0 and m == 1 and S_PER_CORE > 1:
                    units.append(lambda: emit_load_x(1))
            return units

        # ---- schedule: prologue, then involutions with pipelined fillers ----
        emit_load(0)
        emit_load_x(0)
        for u in conv1_units(0):
            u()
        if S_PER_CORE > 1:
            emit_load(1)
        for u in red_units(0):
            u()

        fillers0 = conv1_units(1) + red_units(1) if S_PER_CORE > 1 else []
        for cix in range(NCH):
            for m in range(2):
                emit_inv_chunk(0, cix, m, fillers0)
        for u in fillers0:
            u()

        fillers1 = conv3_units(0)
        if S_PER_CORE > 1:
            for cix in range(NCH):
                for m in range(2):
                    emit_inv_chunk(1, cix, m, fillers1)
        for u in fillers1:
            u()
        if S_PER_CORE > 1:
            for u in conv3_units(1):
                u()

    nc.compile()
    return nc


def _bn_fold(g, b, m, v):
    s = (g / np.sqrt(v + EPS)).astype(np.float32)
    return s, (b - m * s).astype(np.float32)


def _prep_inputs(inputs):
    bf = ml_dtypes.bfloat16
    f32 = np.float32

    s1, t1 = _bn_fold(inputs["bn1_g"], inputs["bn1_b"], inputs["bn1_m"], inputs["bn1_v"])
    t1 = t1 + s1 * inputs["b1"]
    sr, tr = _bn_fold(inputs["red_bn_g"], inputs["red_bn_b"], inputs["red_bn_m"], inputs["red_bn_v"])
    tr = tr + sr * inputs["red_b"]
    s2, t2 = _bn_fold(inputs["bn2_g"], inputs["bn2_b"], inputs["bn2_m"], inputs["bn2_v"])
    s3, t3 = _bn_fold(inputs["bn3_g"], inputs["bn3_b"], inputs["bn3_m"], inputs["bn3_v"])
    t3 = t3 + s3 * inputs["b3"]

    w1t = np.ascontiguousarray(
        inputs["w1"].T.reshape(2, 128, 256).transpose(1, 0, 2)
    ).astype(bf)
    rwt = np.ascontiguousarray(
        inputs["red_w"].T.reshape(2, 128, 64).transpose(1, 0, 2)
    ).astype(bf)
    w3t = np.ascontiguousarray(
        inputs["w3"].T.reshape(2, 128, 256).transpose(1, 0, 2)
    ).astype(bf)

    # span weights replicated x16 over group channels:
    # srep[e, m, p*128 + q] = span_w[(8m + q//16)*49 + p, e]
    sw = inputs["span_w"].reshape(G, 49, RED)             # [g, p, e]
    t = sw.transpose(2, 0, 1).reshape(RED, 2, 8, 49)      # [e, m, g', p]
    rep = np.repeat(t[:, :, :, None, :], GC, axis=3)      # [e, m, g', c, p]
    rep = rep.transpose(0, 1, 4, 2, 3).reshape(RED, 2, 49, 128)  # [e, m, p, q]
    srep = np.ascontiguousarray(rep.reshape(RED, 2, 49 * 128)).astype(bf)

    sb = inputs["span_b"].reshape(G, 49).reshape(2, 8, 49)
    brep = np.repeat(sb[:, :, None, :], GC, axis=2).reshape(2, 128, 49)
    brep = np.ascontiguousarray(brep.transpose(1, 0, 2)).astype(f32)

    scal = np.stack([s1, t1, s2, t2, s3, t3], axis=-1)
    scal = np.ascontiguousarray(scal.reshape(2, 128, 6).transpose(1, 0, 2)).astype(f32)
    scred = np.concatenate(
        [np.stack([sr, tr], axis=-1)] * 2, axis=0
    ).astype(f32)                                         # [128, 2]

    ident = np.eye(128, dtype=f32).astype(bf)

    x = inputs["x"].reshape(B, 2, 128, HW).astype(f32)

    common = dict(w1t=w1t, rwt=rwt, srep=srep, brep=brep, w3t=w3t, scal=scal,
                  scred=scred, ident=ident)
    in_maps = []
    for i in range(N_CORES):
        shard = np.ascontiguousarray(x[i * S_PER_CORE:(i + 1) * S_PER_CORE])
        in_maps.append({**common, "x": shard, "xb": shard.astype(bf)})
    return in_maps


_NC = None


def _get_nc():
    global _NC
    if _NC is None:
        _NC = _build_nc()
    return _NC


def kernel(**inputs):
    inputs = {k: np.asarray(v) for k, v in inputs.items()}
    nc = _get_nc()
    in_maps = _prep_inputs(inputs)
    res = run_bass_kernel_spmd(nc, in_maps, list(range(N_CORES)))
    outs = [res.results[i]["out"].reshape(S_PER_CORE, C, H, W) for i in range(N_CORES)]
    return np.concatenate(outs, axis=0).astype(np.float32)


# revision 15
# speedup vs baseline: 1.0161x; 1.0161x over previous
"""Trainium2 Bass kernel for nn_Block_70944269795510 (involution block).

8 NeuronCores, data-parallel over batch (2 samples/core). Per sample:
  conv1 (PE bf16) -> tanh/bn1 (ACT) into padded-Y bf16 (ypa) + one-element
      shifted parity copy (ypb; keeps bf16 DVE reads 4B-aligned for odd
      kernel-column offsets)
  red (PE bf16) -> relu (ACT) -> rt bf16 [64, HW]
  involution in 4 pixel-chunks of 784 (14 rows), m-serialized, with the
      tap-accumulator living in PSUM:
        per tap: span matmul (PE, 64-contr, 784 cols) -> PSUM
                 evac+bias (ACT identity) -> wd bf16 -> mul wd*ysh (DVE 2x)
                 [STT taps skip the evac: (ps+bias)*ysh via DVE stt at 1x]
                 identity-matmul accumulate pr into acc-PSUM (PE)
  tanh/bn2 (ACT, from acc-PSUM) -> y2 -> conv3 (PE bf16) -> bn3 (ACT)
      -> +x (DVE) -> DMA out

The conv phases of neighbouring samples are software-pipelined into the
involution chunk stream (issued as filler units between taps) so the PE
queue never drains at sample seams - keeping the DVFS p-state high.
"""

import sys

for _p in ("/opt/trn_rl_repo", "/root/.axon_site/_ro/trn_rl_repo"):
    if _p not in sys.path:
        sys.path.append(_p)

import numpy as np
import ml_dtypes
from contextlib import ExitStack

import concourse.bass as bass
import concourse.mybir as mybir
from concourse import bacc
from concourse.tile import TileContext
from concourse.bass_utils import run_bass_kernel_spmd

F32 = mybir.dt.float32
BF16 = mybir.dt.bfloat16
AF = mybir.ActivationFunctionType
ALU = mybir.AluOpType

B, C, H, W = 16, 256, 56, 56
HW = H * W
KK = 7
G, GC, RED = 16, 16, 64
EPS = 1e-5
PAD = 3
N_CORES = 8
S_PER_CORE = B // N_CORES
PW = 64
HP = H + 2 * PAD
NPAD = HP * PW

T448 = 448
CH = 784            # involution pixel-chunk (14 rows of 56)
NCH = 4             # chunks per image
CROWS = 14

# taps whose product is computed by DVE scalar_tensor_tensor straight from
# PSUM (skipping the ACT evacuation); tunable for ACT/DVE balance
STT = frozenset({1, 4, 8, 11, 15, 18, 22, 25, 29, 32, 36, 39, 43, 46})
# taps accumulated on DVE into an SBUF bf16 chain (merged into acc-PSUM at
# the end) instead of PE identity-adds; relieves the PE column budget
DVE_ADD = (0, 9, 19, 28, 37, 45)


def _build_nc():
    nc = bacc.Bacc("TRN2", target_bir_lowering=False, debug=False)

    xd = nc.dram_tensor("x", [S_PER_CORE, 2, 128, HW], F32, kind="ExternalInput").ap()
    xbd = nc.dram_tensor("xb", [S_PER_CORE, 2, 128, HW], BF16, kind="ExternalInput").ap()
    w1d = nc.dram_tensor("w1t", [128, 2, 256], BF16, kind="ExternalInput").ap()
    rwd = nc.dram_tensor("rwt", [128, 2, 64], BF16, kind="ExternalInput").ap()
    spd = nc.dram_tensor("srep", [64, 2, 49 * 128], BF16, kind="ExternalInput").ap()
    brd = nc.dram_tensor("brep", [128, 2, 49], F32, kind="ExternalInput").ap()
    w3d = nc.dram_tensor("w3t", [128, 2, 256], BF16, kind="ExternalInput").ap()
    scd = nc.dram_tensor("scal", [128, 2, 6], F32, kind="ExternalInput").ap()
    srcd = nc.dram_tensor("scred", [128, 2], F32, kind="ExternalInput").ap()
    idd = nc.dram_tensor("ident", [128, 128], BF16, kind="ExternalInput").ap()
    outd = nc.dram_tensor("out", [S_PER_CORE, 2, 128, HW], F32, kind="ExternalOutput").ap()

    with TileContext(nc) as tc, ExitStack() as ctx:
        consts = ctx.enter_context(tc.tile_pool(name="consts", bufs=1))
        xpool = ctx.enter_context(tc.tile_pool(name="xp", bufs=2))
        xbpool = ctx.enter_context(tc.tile_pool(name="xb", bufs=2))
        ypapool = ctx.enter_context(tc.tile_pool(name="ypa", bufs=4))
        ypbpool = ctx.enter_context(tc.tile_pool(name="ypb", bufs=4))
        rpool = ctx.enter_context(tc.tile_pool(name="rp", bufs=2))
        prpool = ctx.enter_context(tc.tile_pool(name="prp", bufs=10))
        y2pool = ctx.enter_context(tc.tile_pool(name="y2p", bufs=4))
        opool = ctx.enter_context(tc.tile_pool(name="op", bufs=1))
        pspool = ctx.enter_context(tc.tile_pool(name="psp", bufs=1, space="PSUM"))

        w1t = consts.tile([128, 2, 256], BF16)
        nc.sync.dma_start(out=w1t, in_=w1d)
        rwt = consts.tile([128, 2, 64], BF16)
        nc.sync.dma_start(out=rwt, in_=rwd)
        srep = consts.tile([64, 2, 49 * 128], BF16)
        nc.sync.dma_start(out=srep, in_=spd)
        brep = consts.tile([128, 2, 49], F32)
        nc.sync.dma_start(out=brep, in_=brd)
        w3t = consts.tile([128, 2, 256], BF16)
        nc.sync.dma_start(out=w3t, in_=w3d)
        scal = consts.tile([128, 2, 6], F32)
        nc.sync.dma_start(out=scal, in_=scd)
        scred = consts.tile([128, 2], F32)
        nc.sync.dma_start(out=scred, in_=srcd)
        ident = consts.tile([128, 128], BF16)
        nc.sync.dma_start(out=ident, in_=idd)

        # 448-col tile pairs packed into [128,1024] psum tiles (512-stripes)
        pairs = [(0, 2), (2, 2), (4, 2), (6, 1)]

        # per-sample state
        ST = [dict() for _ in range(S_PER_CORE)]

        def emit_load(s):
            st = ST[s]
            st["xbs"] = []
            for m in range(2):
                xbt = xbpool.tile([128, HW], BF16, tag="xb", name=f"xb{s}{m}")
                nc.sync.dma_start(out=xbt, in_=xbd[s, m])
                st["xbs"].append(xbt)

        def emit_load_x(s):
            st = ST[s]
            st["xs"] = []
            for m in range(2):
                xt = xpool.tile([128, HW], F32, tag="x", name=f"x{s}{m}")
                nc.sync.dma_start(out=xt, in_=xd[s, m])
                st["xs"].append(xt)

        def emit_conv1_start(s, m):
            st = ST[s]
            if "ypa" not in st:
                st["ypa"], st["ypb"] = [], []
            yat = ypapool.tile([128, NPAD], BF16, tag="ypa", name=f"ya{s}{m}")
            ybt = ypbpool.tile([128, NPAD], BF16, tag="ypb", name=f"yb{s}{m}")
            nc.vector.memset(yat[:], 0.0)
            st["ypa"].append(yat)
            st["ypb"].append(ybt)

        def emit_conv1_pair(s, m, t0, cnt):
            st = ST[s]
            yat = st["ypa"][m]
            yav = yat[:].rearrange("p (h w) -> p h w", w=PW)
            ps = pspool.tile([128, 1024], F32, tag="pss", bufs=3, name=f"c1p{s}{m}{t0}")
            psv = ps[:].rearrange("p (t w) -> p t w", w=512)
            for j in range(cnt):
                t = t0 + j
                for k in range(2):
                    nc.tensor.matmul(
                        psv[:, j, 0:T448],
                        w1t[:, k, m * 128:(m + 1) * 128],
                        st["xbs"][k][:, t * T448:(t + 1) * T448],
                        start=(k == 0), stop=(k == 1),
                    )
            psq = psv[:, 0:cnt, 0:T448].rearrange("p t (r w) -> p t r w", w=W)
            nc.scalar.activation(
                out=yav[:, PAD + 8 * t0:PAD + 8 * (t0 + cnt), PAD:PAD + W]
                .rearrange("p (t r) w -> p t r w", r=8),
                in_=psq,
                func=AF.Tanh,
                scale=scal[:, m, 0:1],
                bias=scal[:, m, 1:2],
            )

        def emit_conv1_end(s, m):
            st = ST[s]
            nc.vector.tensor_copy(st["ypb"][m][:, 0:NPAD - 1], st["ypa"][m][:, 1:NPAD])

        def emit_red_start(s):
            ST[s]["rt"] = rpool.tile([64, HW], BF16, tag="r", name=f"r{s}")

        def emit_red_pair(s, t0, cnt):
            st = ST[s]
            rv = st["rt"][:].rearrange("p (t w) -> p t w", w=T448)
            ps = pspool.tile([128, 1024], F32, tag="pss", bufs=3, name=f"rp{s}{t0}")
            psv = ps[:].rearrange("p (t w) -> p t w", w=512)
            for j in range(cnt):
                t = t0 + j
                for k in range(2):
                    yk = st["ypa"][k][:].rearrange("p (h w) -> p h w", w=PW)
                    rhs = yk[:, PAD + 8 * t:PAD + 8 * (t + 1), PAD:PAD + W]
                    nc.tensor.matmul(
                        psv[0:64, j, 0:T448],
                        rwt[:, k, :],
                        rhs,
                        start=(k == 0), stop=(k == 1),
                    )
            nc.scalar.activation(
                out=rv[:, t0:t0 + cnt, :],
                in_=psv[0:64, 0:cnt, 0:T448],
                func=AF.Relu,
                scale=scred[0:64, 0:1],
                bias=scred[0:64, 1:2],
            )

        def emit_inv_chunk(s, cix, m, fillers):
            """49-tap involution for one (pixel-chunk, m); pops filler
            closures after taps 10/22/34/46 to keep other phases flowing."""
            st = ST[s]
            if "y2" not in st:
                st["y2"] = [
                    y2pool.tile([128, HW], BF16, tag="y2", name=f"y2{s}{mm}")
                    for mm in range(2)
                ]
            c0 = cix * CH
            r0c = cix * CROWS
            yav = st["ypa"][m][:].rearrange("p (h w) -> p h w", w=PW)
            ybv = st["ypb"][m][:].rearrange("p (h w) -> p h w", w=PW)
            rt = st["rt"]
            acc = pspool.tile([128, 1024], F32, tag="acc", bufs=1, name=f"acc{s}{cix}{m}")
            acc_sb = prpool.tile([128, CH], BF16, tag="accsb", bufs=2, name=f"as{s}{cix}{m}")
            pend = {}
            pe_taps = [p for p in range(49) if p not in DVE_ADD]

            def flush_add(k):
                if k in DVE_ADD:
                    if k != DVE_ADD[0]:
                        nc.vector.tensor_add(acc_sb[:], acc_sb[:], pend.pop(k)[:])
                    return
                pr = pend.pop(k)
                first = k == pe_taps[0]
                nc.tensor.matmul(
                    acc[:, 0:512], ident[:], pr[:, 0:512],
                    start=first, stop=False,
                )
                nc.tensor.matmul(
                    acc[:, 512:CH], ident[:], pr[:, 512:CH],
                    start=first, stop=False,
                )

            def emit_tap(p):
                di, dj = p // KK, p % KK
                ps = pspool.tile([128, 1024], F32, tag="pss", bufs=3, name=f"sp{s}{cix}{m}{p}")
                for off, n in ((0, 512), (512, CH - 512)):
                    nc.tensor.matmul(
                        ps[:, off:off + n],
                        srep[:, m, p * 128:(p + 1) * 128],
                        rt[:, c0 + off:c0 + off + n],
                        start=True, stop=True,
                    )
                if dj % 2 == 0:
                    ysh = yav[:, r0c + di:r0c + di + CROWS, dj:dj + W]
                else:
                    ysh = ybv[:, r0c + di:r0c + di + CROWS, dj - 1:dj - 1 + W]
                if p == DVE_ADD[0]:
                    pr = acc_sb
                else:
                    pr = prpool.tile([128, CH], BF16, tag="pr", name=f"pr{s}{cix}{m}{p}")
                if p in STT:
                    nc.vector.scalar_tensor_tensor(
                        out=pr[:].rearrange("p (h w) -> p h w", w=W),
                        in0=ps[:, 0:CH].rearrange("p (h w) -> p h w", w=W),
                        scalar=brep[:, m, p:p + 1],
                        in1=ysh,
                        op0=ALU.add,
                        op1=ALU.mult,
                    )
                else:
                    nc.scalar.activation(
                        out=pr[:],
                        in_=ps[:, 0:CH],
                        func=AF.Identity,
                        scale=1.0,
                        bias=brep[:, m, p:p + 1],
                    )
                    nc.vector.tensor_mul(
                        pr[:].rearrange("p (h w) -> p h w", w=W),
                        pr[:].rearrange("p (h w) -> p h w", w=W),
                        ysh,
                    )
                pend[p] = pr

            # batches of 3 taps: spans together, trailing adds together --
            # each srep<->ident weight switch costs ~105ns of PE drain
            done = 0
            for b in range(0, 51, 3):
                for p in range(b, min(b + 3, 49)):
                    emit_tap(p)
                while done < min(b, 49):
                    flush_add(done)
                    done += 1
                if b in (12, 24, 36, 48) and fillers:
                    fillers.pop(0)()
            while done < 49:
                flush_add(done)
                done += 1
            for off, n, last in ((0, 512, False), (512, CH - 512, True)):
                nc.tensor.matmul(
                    acc[:, off:off + n], ident[:], acc_sb[:, off:off + n],
                    start=False, stop=True,
                )
            nc.scalar.activation(
                out=st["y2"][m][:, c0:c0 + CH],
                in_=acc[:, 0:CH],
                func=AF.Tanh,
                scale=scal[:, m, 2:3],
                bias=scal[:, m, 3:4],
            )

        def emit_conv3_start(s, m):
            st = ST[s]
            if "ot" not in st:
                st["ot"] = {}
            st["ot"][m] = opool.tile([128, HW], F32, tag="o", name=f"o{s}{m}")

        def emit_conv3_pair(s, m, t0, cnt):
            st = ST[s]
            ot = st["ot"][m]
            ov = ot[:].rearrange("p (t w) -> p t w", w=T448)
            ps = pspool.tile([128, 1024], F32, tag="pss", bufs=3, name=f"c3p{s}{m}{t0}")
            psv = ps[:].rearrange("p (t w) -> p t w", w=512)
            for j in range(cnt):
                t = t0 + j
                for k in range(2):
                    nc.tensor.matmul(
                        psv[:, j, 0:T448],
                        w3t[:, k, m * 128:(m + 1) * 128],
                        st["y2"][k][:, t * T448:(t + 1) * T448],
                        start=(k == 0), stop=(k == 1),
                    )
            nc.scalar.activation(
                out=ov[:, t0:t0 + cnt, :],
                in_=psv[:, 0:cnt, 0:T448],
                func=AF.Identity,
                scale=scal[:, m, 4:5],
                bias=scal[:, m, 5:6],
            )

        def emit_conv3_end(s, m):
            st = ST[s]
            ot = st["ot"][m]
            nc.vector.tensor_add(ot[:], ot[:], st["xs"][m][:])
            nc.sync.dma_start(out=outd[s, m], in_=ot[:])

        def conv1_units(s):
            units = []
            for m in range(2):
                units.append(lambda s=s, m=m: emit_conv1_start(s, m))
                for t0, cnt in pairs:
                    units.append(lambda s=s, m=m, t0=t0, cnt=cnt: emit_conv1_pair(s, m, t0, cnt))
                units.append(lambda s=s, m=m: emit_conv1_end(s, m))
            return units

        def red_units(s):
            units = [lambda s=s: emit_red_start(s)]
            for t0, cnt in pairs:
                units.append(lambda s=s, t0=t0, cnt=cnt: emit_red_pair(s, t0, cnt))
            return units

        def conv3_units(s):
            units = []
            for m in range(2):
                units.append(lambda s=s, m=m: emit_conv3_start(s, m))
                for t0, cnt in pairs:
                    units.append(lambda s=s, m=m, t0=t0, cnt=cnt: emit_conv3_pair(s, m, t0, cnt))
                units.append(lambda s=s, m=m: emit_conv3_end(s, m))
                if s == 0 and m == 1 and S_PER_CORE > 1:
                    units.append(lambda: emit_load_x(1))
            return units

        # ---- schedule: prologue, then involutions with pipelined fillers ----
        emit_load(0)
        emit_load_x(0)
        for u in conv1_units(0):
            u()
        if S_PER_CORE > 1:
            emit_load(1)
        for u in red_units(0):
            u()

        fillers0 = conv1_units(1) + red_units(1) if S_PER_CORE > 1 else []
        for cix in range(NCH):
            for m in range(2):
                emit_inv_chunk(0, cix, m, fillers0)
        for u in fillers0:
            u()

        fillers1 = conv3_units(0)
        if S_PER_CORE > 1:
            for cix in range(NCH):
                for m in range(2):
                    emit_inv_chunk(1, cix, m, fillers1)
        for u in fillers1:
            u()
        if S_PER_CORE > 1:
            for u in conv3_units(1):
                u()

    nc.compile()
    return nc


def _bn_fold(g, b, m, v):
    s = (g / np.sqrt(v + EPS)).astype(np.float32)
    return s, (b - m * s).astype(np.float32)


def _prep_inputs(inputs):
    bf = ml_dtypes.bfloat16
    f32 = np.float32

    s1, t1 = _bn_fold(inputs["bn1_g"], inputs["bn1_b"], inputs["bn1_m"], inputs["bn1_v"])
    t1 = t1 + s1 * inputs["b1"]
    sr, tr = _bn_fold(inputs["red_bn_g"], inputs["red_bn_b"], inputs["red_bn_m"], inputs["red_bn_v"])
    tr = tr + sr * inputs["red_b"]
    s2, t2 = _bn_fold(inputs["bn2_g"], inputs["bn2_b"], inputs["bn2_m"], inputs["bn2_v"])
    s3, t3 = _bn_fold(inputs["bn3_g"], inputs["bn3_b"], inputs["bn3_m"], inputs["bn3_v"])
    t3 = t3 + s3 * inputs["b3"]

    w1t = np.ascontiguousarray(
        inputs["w1"].T.reshape(2, 128, 256).transpose(1, 0, 2)
    ).astype(bf)
    rwt = np.ascontiguousarray(
        inputs["red_w"].T.reshape(2, 128, 64).transpose(1, 0, 2)
    ).astype(bf)
    w3t = np.ascontiguousarray(
        inputs["w3"].T.reshape(2, 128, 256).transpose(1, 0, 2)
    ).astype(bf)

    # span weights replicated x16 over group channels:
    # srep[e, m, p*128 + q] = span_w[(8m + q//16)*49 + p, e]
    sw = inputs["span_w"].reshape(G, 49, RED)             # [g, p, e]
    t = sw.transpose(2, 0, 1).reshape(RED, 2, 8, 49)      # [e, m, g', p]
    rep = np.repeat(t[:, :, :, None, :], GC, axis=3)      # [e, m, g', c, p]
    rep = rep.transpose(0, 1, 4, 2, 3).reshape(RED, 2, 49, 128)  # [e, m, p, q]
    srep = np.ascontiguousarray(rep.reshape(RED, 2, 49 * 128)).astype(bf)

    sb = inputs["span_b"].reshape(G, 49).reshape(2, 8, 49)
    brep = np.repeat(sb[:, :, None, :], GC, axis=2).reshape(2, 128, 49)
    brep = np.ascontiguousarray(brep.transpose(1, 0, 2)).astype(f32)

    scal = np.stack([s1, t1, s2, t2, s3, t3], axis=-1)
    scal = np.ascontiguousarray(scal.reshape(2, 128, 6).transpose(1, 0, 2)).astype(f32)
    scred = np.concatenate(
        [np.stack([sr, tr], axis=-1)] * 2, axis=0
    ).astype(f32)                                         # [128, 2]

    ident = np.eye(128, dtype=f32).astype(bf)

    x = inputs["x"].reshape(B, 2, 128, HW).astype(f32)

    common = dict(w1t=w1t, rwt=rwt, srep=srep, brep=brep, w3t=w3t, scal=scal,
                  scred=scred, ident=ident)
    in_maps = []
    for i in range(N_CORES):
        shard = np.ascontiguousarray(x[i * S_PER_CORE:(i + 1) * S_PER_CORE])
        in_maps.append({**common, "x": shard, "xb": shard.astype(bf)})
    return in_maps


_NC = None


def _get_nc():
    global _NC
    if _NC is None:
        _NC = _build_nc()
    return _NC


def kernel(**inputs):
    inputs = {k: np.asarray(v) for k, v in inputs.items()}
    nc = _get_nc()
    in_maps = _prep_inputs(inputs)
    res = run_bass_kernel_spmd(nc, in_maps, list(range(N_CORES)))
    outs = [res.results[i]["out"].reshape(S_PER_CORE, C, H, W) for i in range(N_CORES)]
    return np.concatenate(outs, axis=0).astype(np.float32)
